# revision 7
# baseline (speedup 1.0000x reference)
"""Bahdanau attention kernel for 8 Trainium2 NeuronCores (v2).

Problem (hardcoded shapes): B=32, T=8192, D_ENC=256, D_HID=512, D_ATT=512.
    proj = encoder_out @ w1 + b1 + (h @ w2 + b2) + (c @ w3 + b3)   # [B,T,512]
    scores = tanh(proj) @ wv (+ bv)                                # [B,T,1]
    attn = softmax(scores, axis=T)
    context = sum_t attn * encoder_out                             # [B,256]

Sharding: data-parallel over batch, 4 batches per core, no collectives.

v2 design (vs the bf16 baseline):
  - Projection runs in fp8 DoubleRow (K=256 packed into 128 rows), halving
    the PE streaming time.  encoder fp8 at scale 1; w1 fp8 with a per-j
    scale (j0/j1: 2.0, j2/j3: S from the tanh-poly fit) and CONSTRAINED
    ROUNDING: round-up/down choices per element are flipped host-side to
    cancel the coherent (softmax-biasing) component of the quantization
    error, sum_a wv_a*E[tanh']*dW[d,a], for all 32 batches at once.
  - tanh is split between the ACT engine (exact, fused scale+bias) and a
    custom 8-stage DVE op TANH5_ANT computing
    clamp(((u^2+B)u^2+A)u, -1, 1), u = psum + S*vb  (deg-5 odd minimax fit
    of tanh with leading coeff absorbed into the host-side S prescale;
    vb rides the latched C3/in1 slot per partition).
  - Scores: tanh tiles stay bf16; four col-tiled M=1 matmuls (tile_position
    32j) run concurrently on the PE; the psum bank is copied once per chunk
    to SBUF, the 4 partial rows bounce through DRAM into per-batch column
    tiles, and cheap strided [128,8,4] DVE adds reduce over j per half
    batch before the ACT exp.
  - Pass-B (context accumulation) is split between the PE (e-column
    stationary matmuls accumulating a [1,257] psum row per batch) and DVE
    scalar_tensor_tensor accumulators; the ones-column of encN yields the
    softmax normalizer for free.  The last batch runs entirely on the PE
    with per-chunk exp so the kernel tail stays short.
"""

import os
import sys

for _p in ("/opt/trn_rl_repo", "/root/.axon_site", "/root/.axon_site/_ro/pypackages"):
    if os.path.isdir(_p) and _p not in sys.path:
        sys.path.append(_p)

import numpy as np
import ml_dtypes

import concourse.bass as bass
import concourse.tile as tile
from concourse import bacc, bass_isa, mybir
from concourse.bass_utils import run_bass_kernel_spmd

BF16 = ml_dtypes.bfloat16
FP8 = ml_dtypes.float8_e4m3

B, T, D_ENC, D_HID, D_ATT = 32, 8192, 256, 512, 512
N_CORES = 8
BPC = B // N_CORES          # batches per core = 4
P = 128                     # partitions
TC = 512                    # pass-A chunk (timesteps)
NCH = T // TC               # chunks per batch = 16
KD = D_ENC // P             # contraction subtiles = 2
NJ = D_ATT // P             # a-blocks = 4
DE1 = D_ENC + 1             # encN row with ones column = 257
NCOL = T // P               # e columns per batch = 64
GPC = TC // P               # pass-B groups per chunk = 4

# tanh deg-5 fit: tanh(z) ~= clamp(((u^2 + PB)u^2 + PA)u, -1, 1), u = PS*z
PA = 2.040669042070581
PB = -2.022485437340684
PS = 0.46830810611420903
NU_ACT = 2.0                # w1 fp8 scale for the always-ACT j-blocks (0, 1)

# DVE-owned j-blocks per chunk; j2 alternates so ~62% of tiles go to ACT
def _dve_js(i):
    return (2, 3) if i % 4 < 3 else (3,)

# pass-B group -> engine (batches 0..2; batch 3 is all-PE for the tail)
def _group_on_dve(b, g):
    return b < BPC - 1 and g % 8 == 2

_PROGRAM_CACHE = {}

# --------------------------------------------------------------------------
# custom DVE op: clamp(((u^2 + s1)u^2 + imm2)u, s0, 1), u = in0 + latch(in1)
# --------------------------------------------------------------------------


def _register_tanh5():
    from concourse.dve_ops import (
        OPS, DveOp, CUSTOM_DVE_SPECS, _CUSTOM_DVE_ROW_BASE,
        _SUB_OPCODE_FOR_NAME,
    )
    from concourse.dve_spec import (
        Spec, Src0, C0, C1, C2, C3, One, maxx, minn, lower, _has_src1,
        _spill_c3_to_src1,
    )
    from concourse.dve_uop import DveOpSpec
    from concourse.dve_table_gen import dve_ver_for

    for op in OPS:
        if op.name == "TANH5_ANT":
            return op
    u = Src0 + C3
    t = u * u
    w = ((t + C1) * t + C2) * u
    body = _spill_c3_to_src1(minn(maxx(w, C0), One))

    def _ref(in0, in1, s0, s1, imm2):
        uu = in0.astype(np.float32) + in1
        tt = uu * uu
        return np.clip(((tt + s1) * tt + imm2) * uu, s0, 1.0)

    spec = Spec(body=body, reference=_ref)
    ver = dve_ver_for("TRN2")
    uops = lower(spec, ver=ver)
    opcode = _CUSTOM_DVE_ROW_BASE + len(OPS)
    sha = DveOpSpec(name="TANH5_ANT", opcode=opcode, uops=uops,
                    rd1_en=_has_src1(spec)).sha(ver)
    op = DveOp("TANH5_ANT", spec, subdim=False, uops_sha={ver: sha})
    OPS.append(op)
    _SUB_OPCODE_FOR_NAME["TANH5_ANT"] = opcode
    CUSTOM_DVE_SPECS["TANH5_ANT"] = spec
    return op


TANH5 = _register_tanh5()


# --------------------------------------------------------------------------
# program
# --------------------------------------------------------------------------


def _build_program():
    if "nc" in _PROGRAM_CACHE:
        return _PROGRAM_CACHE["nc"]

    f32 = mybir.dt.float32
    bf16 = mybir.dt.bfloat16
    fp8 = mybir.dt.float8e4
    Act = mybir.ActivationFunctionType
    Alu = mybir.AluOpType
    DR = mybir.MatmulPerfMode.DoubleRow

    nc = bacc.Bacc("TRN2", target_bir_lowering=False, debug=False,
                   num_devices=N_CORES)

    encT = nc.dram_tensor("encT", [BPC, D_ENC, T], fp8, kind="ExternalInput")
    encN = nc.dram_tensor("encN", [BPC, T, DE1], bf16, kind="ExternalInput")
    # w1 packed for DoubleRow: [ki, ko, j, m] = w1q[ko*128+ki, j*128+m]
    w1t = nc.dram_tensor("w1t", [P, KD, NJ, P], fp8, kind="ExternalInput")
    wvt = nc.dram_tensor("wvt", [P, NJ], bf16, kind="ExternalInput")
    vbt = nc.dram_tensor("vbt", [P, BPC * NJ], f32, kind="ExternalInput")
    vbs = nc.dram_tensor("vbs", [P, BPC * NJ], f32, kind="ExternalInput")
    outd = nc.dram_tensor("out", [BPC, D_ENC], f32, kind="ExternalOutput")
    sscr = nc.dram_tensor("sscr", [BPC * NCH, NJ, TC], f32)

    with tile.TileContext(nc) as tc:
        import contextlib
        with contextlib.ExitStack() as ctx:
            const = ctx.enter_context(tc.tile_pool(name="const", bufs=1))
            encT_pool = ctx.enter_context(tc.tile_pool(name="encT", bufs=6))
            encN_pool = ctx.enter_context(tc.tile_pool(name="encN", bufs=8))
            tanh_pool = ctx.enter_context(tc.tile_pool(name="tanh", bufs=10))
            scs_pool = ctx.enter_context(tc.tile_pool(name="scs", bufs=2))
            stg_pool = ctx.enter_context(tc.tile_pool(name="stg", bufs=2))
            tmp_pool = ctx.enter_context(tc.tile_pool(name="tmp", bufs=4))
            e_pool = ctx.enter_context(tc.tile_pool(name="e", bufs=2))
            sm_pool = ctx.enter_context(tc.tile_pool(name="sm", bufs=4))
            osb_pool = ctx.enter_context(tc.tile_pool(name="osb", bufs=2))
            acc1_pool = ctx.enter_context(tc.tile_pool(name="acc1", bufs=2))
            acc2_pool = ctx.enter_context(tc.tile_pool(name="acc2", bufs=2))
            hid_psum = ctx.enter_context(
                tc.tile_pool(name="hid", bufs=5, space="PSUM"))
            sc_psum = ctx.enter_context(
                tc.tile_pool(name="sc", bufs=2, space="PSUM"))
            cf_psum = ctx.enter_context(
                tc.tile_pool(name="cfin", bufs=1, space="PSUM"))

            # constants
            w1_sb = const.tile([P, KD, NJ, P], fp8)
            nc.scalar.dma_start(w1_sb[:], w1t[:])
            wv_sb = const.tile([P, NJ], bf16)
            nc.scalar.dma_start(wv_sb[:], wvt[:])
            vbt_sb = const.tile([P, BPC * NJ], f32)
            nc.scalar.dma_start(vbt_sb[:], vbt[:])
            vbs_sb = const.tile([P, BPC * NJ], f32)
            nc.scalar.dma_start(vbs_sb[:], vbs[:])
            ones128 = const.tile([P, 1], f32)
            nc.gpsimd.memset(ones128[:], 1.0)

            stage = {}    # batch -> [128, 256] f32 scattered partial cols
            e_sb = {}     # batch -> [128, 64] bf16 exp(scores)
            acc1 = {}     # batch -> [128, 257] f32 DVE accumulator (even)
            acc2 = {}     # batch -> [128, 257] f32 DVE accumulator (odd)
            cfp = {}      # batch -> [1, 257] psum row (PE accumulator)
            cf_open = {}  # batch -> whether cfp already has a start matmul
            tanh_of = {}  # chunk -> list of 4 tanh tiles
            copy_flip = [0]

            def emit_A_main(b, i):
                encT_t = encT_pool.tile([P, KD, TC], fp8)
                src_ap = (encT[b, :, i * TC:(i + 1) * TC]
                          .rearrange("(k p) t -> p k t", p=P))
                if b == 0 and i == 0:
                    for k in range(KD):
                        nc.sync.dma_start(encT_t[:, k, :], src_ap[:, k, :])
                else:
                    nc.sync.dma_start(encT_t[:], src_ap)
                tiles = []
                djs = _dve_js(i)
                for j in range(NJ):
                    h_ps = hid_psum.tile([P, TC], f32, tag="hid")
                    nc.tensor.matmul(h_ps[:], w1_sb[:, :, j, :], encT_t[:],
                                     start=True, stop=True, perf_mode=DR)
                    tt = tanh_pool.tile([P, TC], bf16, tag="tanh")
                    col = b * NJ + j
                    if j in djs:
                        nc.vector._custom_dve(
                            TANH5, out=tt[:], in0=h_ps[:],
                            in1=vbs_sb[:, col:col + 1],
                            s0=-1.0, s1=PB, imm2=PA)
                    else:
                        nc.scalar.activation(
                            tt[:], h_ps[:], Act.Tanh,
                            bias=vbt_sb[:, col:col + 1],
                            scale=1.0 / (NU_ACT if j < 2 else PS))
                    tiles.append(tt)
                tanh_of[i] = tiles

            def emit_A_scores(b, i):
                tiles = tanh_of.pop(i)
                sc_ps = sc_psum.tile([P, TC], f32, tag="sc")
                for j in range(NJ):
                    nc.tensor.matmul(
                        sc_ps[32 * j:32 * j + 1, :],
                        wv_sb[:, j:j + 1], tiles[j][:],
                        start=True, stop=True, tile_position=(0, 32 * j))
                sc_sb = scs_pool.tile([P, TC], f32, tag="scs")
                # alternate the bank copy between ACT and DVE
                if copy_flip[0] % 2 == 0:
                    nc.scalar.activation(sc_sb[:], sc_ps[:], Act.Copy)
                else:
                    nc.vector.tensor_copy(sc_sb[:], sc_ps[:])
                copy_flip[0] += 1
                row = sscr[b * NCH + i]
                nc.gpsimd.dma_start(
                    row, sc_sb[:].rearrange("(j o) n -> j o n", o=32)[:, 0, :])
                nc.gpsimd.dma_start(
                    stage[b][:, i * NJ * GPC:(i + 1) * NJ * GPC],
                    row.rearrange("j (u p) -> p (j u)", p=P))
                if b == BPC - 1 and i >= NCH // 2:
                    emit_combine_exp_chunk(b, i)

            def _stage_view(b, ih, j, nchunk=8, i0=None):
                # [128, nchunk, 4] AP over stage cols i*16 + j*4 + u
                i0 = ih * 8 if i0 is None else i0
                v = stage[b][:].rearrange("p (i j u) -> p i (j u)", j=NJ, u=GPC)
                return v[:, i0:i0 + nchunk, 4 * j:4 * j + 4]

            def emit_combine_exp(b, ih):
                t01 = tmp_pool.tile([P, 8, GPC], f32, tag="t01")
                t23 = tmp_pool.tile([P, 8, GPC], f32, tag="t23")
                sc = tmp_pool.tile([P, 8, GPC], f32, tag="scol")
                nc.vector.tensor_add(t01[:], _stage_view(b, ih, 0),
                                     _stage_view(b, ih, 1))
                nc.vector.tensor_add(t23[:], _stage_view(b, ih, 2),
                                     _stage_view(b, ih, 3))
                nc.vector.tensor_add(sc[:], t01[:], t23[:])
                nc.scalar.activation(
                    e_sb[b][:, ih * 32:(ih + 1) * 32],
                    sc[:].rearrange("p i u -> p (i u)"), Act.Exp)

            def emit_combine_exp_chunk(b, i):
                t01 = tmp_pool.tile([P, 1, GPC], f32, tag="t01")
                t23 = tmp_pool.tile([P, 1, GPC], f32, tag="t23")
                sc = tmp_pool.tile([P, 1, GPC], f32, tag="scol")
                nc.vector.tensor_add(t01[:], _stage_view(b, 0, 0, 1, i),
                                     _stage_view(b, 0, 1, 1, i))
                nc.vector.tensor_add(t23[:], _stage_view(b, 0, 2, 1, i),
                                     _stage_view(b, 0, 3, 1, i))
                nc.vector.tensor_add(sc[:], t01[:], t23[:])
                nc.scalar.activation(
                    e_sb[b][:, i * GPC:(i + 1) * GPC],
                    sc[:].rearrange("p i u -> p (i u)"), Act.Exp)

            def emit_batch_init(b):
                stage[b] = stg_pool.tile([P, NCH * NJ * GPC], f32,
                                         tag="stg", name=f"stage{b}")
                e_sb[b] = e_pool.tile([P, NCOL], bf16, tag="e",
                                      name=f"e{b}")

            def emit_acc_init(b):
                acc1[b] = acc1_pool.tile([P, DE1], f32, tag="a1",
                                         name=f"acc1_{b}")
                nc.gpsimd.memset(acc1[b][:], 0.0)
                acc2[b] = acc2_pool.tile([P, DE1], f32, tag="a2",
                                         name=f"acc2_{b}")
                nc.gpsimd.memset(acc2[b][:], 0.0)

            def emit_B_slot(b, g0, n):
                """n pass-B groups [g0, g0+n) of batch b (one encN DMA)."""
                encN_t = encN_pool.tile([P, n, DE1], bf16)
                nc.gpsimd.dma_start(
                    encN_t[:],
                    encN[b, g0 * P:(g0 + n) * P, :]
                        .rearrange("(n p) d -> p n d", p=P))
                dve_par = 0
                for gi in range(n):
                    g = g0 + gi
                    if _group_on_dve(b, g):
                        acc = acc1 if dve_par % 2 == 0 else acc2
                        dve_par += 1
                        nc.vector.scalar_tensor_tensor(
                            acc[b][:], encN_t[:, gi, :],
                            e_sb[b][:, g:g + 1], acc[b][:],
                            op0=Alu.mult, op1=Alu.add)
                    else:
                        if b not in cfp:
                            cfp[b] = cf_psum.tile([1, DE1], f32, tag="cfin",
                                                  name=f"cf{b}")
                            cf_open[b] = False
                        nc.tensor.matmul(
                            cfp[b][:], e_sb[b][:, g:g + 1], encN_t[:, gi, :],
                            start=not cf_open[b], stop=False)
                        cf_open[b] = True

            def emit_B_finalize(b):
                cf = cfp[b]
                nc.tensor.matmul(cf[:], ones128[:], acc1[b][:],
                                 start=False, stop=False)
                nc.tensor.matmul(cf[:], ones128[:], acc2[b][:],
                                 start=False, stop=True)
                rzb = sm_pool.tile([1, 1], f32, tag="rz", name=f"rz{b}")
                nc.vector.reciprocal(rzb[:], cf[:, D_ENC:D_ENC + 1])
                o_sb = osb_pool.tile([1, D_ENC], f32, tag="osb")
                nc.vector.tensor_scalar_mul(o_sb[:], cf[:, 0:D_ENC], rzb[:])
                nc.sync.dma_start(outd[b:b + 1, :], o_sb[:])
                del cfp[b]

            # first-half groups (32) distributed over slots 9..15
            FH = [5, 5, 5, 5, 4, 4, 4]
            FH_OFF = [0, 5, 10, 15, 20, 24, 28]

            for step in range(BPC + 1):
                for i in range(NCH):
                    if step < BPC and i == 0:
                        emit_batch_init(step)
                    if i < 8:
                        if step >= 1:
                            b = step - 1
                            emit_B_slot(b, 32 + 4 * i, 4)
                            if i == 7:
                                emit_B_finalize(b)
                    elif i >= 9 and step < BPC:
                        k = i - 9
                        if FH[k]:
                            emit_B_slot(step, FH_OFF[k], FH[k])
                    if step < BPC:
                        if i == 8:
                            emit_acc_init(step)
                        if i == 9:
                            emit_combine_exp(step, 0)
                        emit_A_main(step, i)
                        if i > 0:
                            emit_A_scores(step, i - 1)
                if step < BPC:
                    emit_A_scores(step, NCH - 1)
                    if step != BPC - 1:
                        emit_combine_exp(step, 1)

    nc.finalize()
    _PROGRAM_CACHE["nc"] = nc
    return nc


# --------------------------------------------------------------------------
# host-side prep
# --------------------------------------------------------------------------


def _fp8_round_pair(x):
    lo = x.astype(FP8).astype(np.float32)
    up = np.nextafter(lo.astype(FP8), np.float32(np.inf).astype(FP8)
                      ).astype(np.float32)
    dn = np.nextafter(lo.astype(FP8), np.float32(-np.inf).astype(FP8)
                      ).astype(np.float32)
    other = np.where(x > lo, up, np.where(x < lo, dn, lo))
    return lo, other


def _constrained_quant(wcol, nu, hb_w):
    """fp8(wcol*nu) with roundings flipped to cancel the coherent
    per-batch error  R[d, b] = sum_a hb_w[b, a] * (q[d,a]/nu - w[d,a])."""
    NA = wcol.shape[1]
    x = wcol * nu
    near, other = _fp8_round_pair(x)
    dW_near = near / nu - wcol
    flipdelta = (other - near) / nu
    R = dW_near @ hb_w.T
    q = near.copy()
    for d0 in range(0, wcol.shape[0], 16):
        sl = slice(d0, d0 + 16)
        imp = flipdelta[sl][:, :, None] * hb_w.T[None, :, :]
        Rc = R[sl].copy()
        flipped = np.zeros((16, NA), bool)
        for _ in range(200):
            newR = Rc[:, None, :] + imp
            cost = (newR ** 2).sum(-1)
            cost[flipped] = np.inf
            amin = cost.argmin(1)
            cur = (Rc ** 2).sum(-1)
            gains = cur - cost[np.arange(16), amin]
            upd = gains > 1e-18
            if not upd.any():
                break
            for r in np.where(upd)[0]:
                a = amin[r]
                Rc[r] += imp[r, a]
                q[d0 + r, a] = other[d0 + r, a]
                flipped[r, a] = True
    return q


def _prep_inputs(encoder_out, hidden_state_h, hidden_state_c,
                 w1, b1, w2, b2, w3, b3, wv, bv):
    enc = np.asarray(encoder_out, dtype=np.float32)
    w1f = np.asarray(w1, np.float32)
    wvf = np.asarray(wv, np.float32).reshape(-1)
    vb = (np.asarray(b1, np.float32)
          + np.asarray(hidden_state_h, np.float32) @ np.asarray(w2, np.float32)
          + np.asarray(b2, np.float32)
          + np.asarray(hidden_state_c, np.float32) @ np.asarray(w3, np.float32)
          + np.asarray(b3, np.float32))                        # [B, 512]
    # bv cancels in softmax; dropped.

    # E[tanh'(z)] per (batch, a) under z ~ N(vb, ||w1[:,a]||^2)
    gh_x, gh_w = np.polynomial.hermite_e.hermegauss(41)
    sig = np.sqrt((w1f ** 2).sum(0))
    zz = vb[..., None] + sig[None, :, None] * gh_x
    hbar = ((1 - np.tanh(zz) ** 2) * gh_w).sum(-1) / gh_w.sum()
    hb_w = wvf * hbar                                          # [B, 512]

    w1q = np.empty_like(w1f)
    for j in range(NJ):
        sl = slice(P * j, P * (j + 1))
        nu = NU_ACT if j < 2 else PS
        w1q[:, sl] = _constrained_quant(w1f[:, sl], nu, hb_w[:, sl])
    # pack [ki, ko, j, m] = w1q_scaled[ko*128 + ki, j*128 + m]
    w1_h = np.ascontiguousarray(
        w1q.reshape(KD, P, NJ, P).transpose(1, 0, 2, 3)).astype(FP8)

    wv_h = np.ascontiguousarray(
        wvf.reshape(NJ, P).T).astype(BF16)                     # [128, 4]

    in_maps = []
    for c in range(N_CORES):
        slb = slice(c * BPC, (c + 1) * BPC)
        enc_c = enc[slb]
        encT_c = np.ascontiguousarray(enc_c.transpose(0, 2, 1)).astype(FP8)
        encN_c = np.ascontiguousarray(np.concatenate(
            [enc_c, np.ones((BPC, T, 1), np.float32)], axis=2)).astype(BF16)
        vb_c = vb[slb]                                         # [4, 512]
        vbt_c = np.ascontiguousarray(
            vb_c.reshape(BPC, NJ, P).transpose(2, 0, 1).reshape(P, BPC * NJ)
        ).astype(np.float32)
        in_maps.append({
            "encT": encT_c,
            "encN": encN_c,
            "w1t": w1_h,
            "wvt": wv_h,
            "vbt": vbt_c,
            "vbs": (vbt_c * np.float32(PS)).astype(np.float32),
        })
    return in_maps


def kernel(**inputs):
    nc = _build_program()
    in_maps = _prep_inputs(**inputs)
    res = run_bass_kernel_spmd(nc, in_maps, list(range(N_CORES)))
    out = np.concatenate([res.results[c]["out"] for c in range(N_CORES)],
                         axis=0)
    return out.astype(np.float32)


if __name__ == "__main__":
    rng = np.random.default_rng(0)
    ins = {
        "encoder_out": rng.standard_normal((B, T, D_ENC), dtype=np.float32),
        "hidden_state_h": rng.standard_normal((B, D_HID), dtype=np.float32),
        "hidden_state_c": rng.standard_normal((B, D_HID), dtype=np.float32),
        "w1": (rng.standard_normal((D_ENC, D_ATT), dtype=np.float32)
               / np.sqrt(D_ENC)),
        "b1": np.zeros(D_ATT, np.float32),
        "w2": (rng.standard_normal((D_HID, D_ATT), dtype=np.float32)
               / np.sqrt(D_HID)),
        "b2": np.zeros(D_ATT, np.float32),
        "w3": (rng.standard_normal((D_HID, D_ATT), dtype=np.float32)
               / np.sqrt(D_HID)),
        "wv": (rng.standard_normal((D_ATT, 1), dtype=np.float32)
               / np.sqrt(D_ATT)),
        "b3": np.zeros(D_ATT, np.float32),
        "bv": np.zeros(1, np.float32),
    }
    got = kernel(**ins)
    print("kernel output:", got.shape, got.dtype)
